# revision 1
# baseline (speedup 1.0000x reference)
"""GCNConv Trainium2 kernel: out = A_norm @ (X @ W) == (A_norm @ X) @ W.

Self-contained: shards the graph across 8 NeuronCores (1D row partition of
destination rows), runs a Bass/Tile kernel per core via
bass_utils.run_bass_kernel_spmd, and reassembles the full output.

Per-core device kernel:
  - X (full [n_nodes, 128] fp16) stays in DRAM; source rows are gathered with
    the dma_gather custom DMA (int16 indices) from nsub subtable slices of
    <32768 rows each.
  - Destination tiles = 128 rows. Groups of `gt` tiles share one gather call
    per subtable pass; each pass gets its own SBUF slab so the passes'
    transfers overlap.
  - Segment-sum on the PE: psumT[:, r0:r0+w] += G_chunk.T @ rhs_seg, where
    rhs_seg [128, w] holds degrees[e] at (slot, local_row[e]); rhs is built
    on-chip as (rowloc == iota) * deg from two small per-edge tensors.
  - psumT is Y^T [feat, row]; cast to fp16 it becomes the stationary operand
    of the final weights matmul: out_tile = (Y^T).T @ W.

The program structure is rebuilt per input (data-dependent segment windows)
but is identical across the 8 cores: per-(group,pass) stream lengths are
padded to the max over cores and segment windows are the union over cores.
"""

import numpy as np
from dataclasses import dataclass, field

import concourse.bass as bass
import concourse.bacc as bacc
import concourse.tile as tile
from concourse import mybir
from concourse.bass_utils import run_bass_kernel_spmd
from concourse.vector_clock import ScopedClock

F16 = mybir.dt.float16
F32 = mybir.dt.float32
I16 = mybir.dt.int16

WIN = 32          # psum column window per segment
SENT = 1024.0     # rowloc sentinel (never equals iota 0..WIN-1)
NCORES = 8
GT = 4            # tiles per gather group
NSUB = 4          # subtable passes (int16 index range)

# ---------------------------------------------------------------------------
# Patch TileContext for walrus builds that reject >1 sync-wait/instruction.
# ---------------------------------------------------------------------------

_orig_commit = tile.TileContext._commit_instruction


def _commit_patched(self, inst, lazy_reg_writes: bool = True):
    si = getattr(inst, "sync_info", None)
    if (si is not None and si.on_wait and len(si.on_wait) > 1
            and inst.engine != mybir.EngineType.Unassigned):
        waits = list(si.on_wait)
        imm = [w for w in waits if w.wait_mode == "sem-ge-imm"]
        other = [w for w in waits if w.wait_mode != "sem-ge-imm"]
        assert len(other) <= 1, f"cannot split reg-waits: {waits}"
        keep = other if other else imm[:1]
        hoist = imm if other else imm[1:]
        inst.sync_info = mybir.SyncInfo(on_wait=list(keep),
                                        on_update=list(si.on_update or []))
        for w in hoist:
            nop = mybir.InstNoOp(name=self.nc.get_next_instruction_name(),
                                 ins=[], outs=[])
            nop.engine = inst.engine
            nop.bass_nofuse = True
            nop.sync_info = mybir.SyncInfo(on_wait=[w], on_update=[])
            _orig_commit(self, nop, lazy_reg_writes=False)
    return _orig_commit(self, inst, lazy_reg_writes)


def _drain_and_barrier_patched(self, tick_clock, wait_clock):
    nc = self.nc
    probe = nc.sync.nop(nofuse=True)
    wait_clock.add_sem_waits(probe.ins, ScopedClock({None: tick_clock.global_clock}))
    si = probe.ins.sync_info
    waits = list(si.on_wait) if si is not None and si.on_wait else []
    if waits:
        probe.ins.sync_info = mybir.SyncInfo(on_wait=waits[:1], on_update=[])
        for w in waits[1:]:
            n = nc.sync.nop(nofuse=True)
            n.ins.sync_info = mybir.SyncInfo(on_wait=[w], on_update=[])
    nc.sync.drain()
    nc.all_engine_barrier()
    assert self.sems is not None
    popped = nc._tile_sem_poison_stack.pop()
    assert popped is self._sem_poison
    nc.clear_and_free_semaphores(list(self.sems.allocated().values()))
    nc.all_engine_barrier()


tile.TileContext._commit_instruction = _commit_patched
tile.TileContext._drain_and_barrier = _drain_and_barrier_patched

# ---------------------------------------------------------------------------
# Host-side prep
# ---------------------------------------------------------------------------


@dataclass
class Seg:
    pass_id: int
    chunk: int
    r0: int
    w: int
    scol: int = -1


@dataclass
class Call:
    pass_id: int
    idx_ofs: int
    n_idx: int


@dataclass
class Group:
    calls: list = field(default_factory=list)
    tiles: list = field(default_factory=list)
    segs: dict = field(default_factory=dict)
    idx_ofs: int = 0
    idx_w: int = 0


def shard_edges(row_pointers, column_index, degrees, ncores=NCORES):
    rp = row_pointers.astype(np.int64)
    n_total_rows = rp.shape[0] - 1
    rows_per_core = (n_total_rows + ncores - 1) // ncores
    n_edges = column_index.shape[0]
    # reference semantics: rows = clip(searchsorted(rp, e, 'right') - 1,
    # 0, n_rows - 1); equivalently below (incl. clip of the tails).
    edge_row = np.minimum(
        np.searchsorted(rp[1:], np.arange(n_edges), side="right"),
        n_total_rows - 1)
    cores = []
    for r in range(ncores):
        r_lo = min(r * rows_per_core, n_total_rows)
        r_hi = min(r_lo + rows_per_core, n_total_rows)
        e_lo, e_hi = np.searchsorted(edge_row, [r_lo, r_hi])
        cores.append(((edge_row[e_lo:e_hi] - r_lo).astype(np.int64),
                      column_index[e_lo:e_hi].astype(np.int64),
                      degrees[e_lo:e_hi].astype(np.float32)))
    return cores, rows_per_core, n_total_rows


def prep_all(cores_edges, n_rows_core, n_nodes, gt=GT, nsub=NSUB):
    ncores = len(cores_edges)
    sub = (n_nodes + nsub - 1) // nsub
    assert sub <= 32767, "subtable must fit int16 indexing"
    n_tiles = (n_rows_core + 127) // 128

    tile_edges = []
    for er, ec, ed in cores_edges:
        e_ofs = np.searchsorted(er, np.arange(0, n_tiles * 128 + 1, 128))
        te = []
        for t in range(n_tiles):
            lo, hi = e_ofs[t], e_ofs[t + 1]
            te.append((er[lo:hi] - t * 128, ec[lo:hi], ed[lo:hi]))
        tile_edges.append(te)

    groups = []
    idx_streams = [[] for _ in range(ncores)]
    row_cols = [[] for _ in range(ncores)]
    deg_cols = [[] for _ in range(ncores)]
    idx_cursor = 0
    scol = 0
    max_pass_chunks = [0] * nsub

    for g0 in range(0, n_tiles, gt):
        g = Group()
        g.tiles = list(range(g0, min(g0 + gt, n_tiles)))
        g.idx_ofs = idx_cursor
        tile_seglists = {t: [] for t in g.tiles}
        for c in range(nsub):
            core_idx, core_row, core_deg, core_til = [], [], [], []
            for k in range(ncores):
                si, sr, sd, st = [], [], [], []
                for t in g.tiles:
                    rl, cl, dl = tile_edges[k][t]
                    m = (cl // sub) == c
                    si.append((cl[m] - c * sub).astype(np.int16))
                    sr.append(rl[m].astype(np.float32))
                    sd.append(dl[m].astype(np.float32))
                    st.append(np.full(int(m.sum()), t, np.int32))
                core_idx.append(np.concatenate(si))
                core_row.append(np.concatenate(sr))
                core_deg.append(np.concatenate(sd))
                core_til.append(np.concatenate(st))
            P = max(ci.size for ci in core_idx)
            P = ((P + 127) // 128) * 128
            if P == 0:
                continue
            nch = P // 128
            max_pass_chunks[c] = max(max_pass_chunks[c], nch)
            for k in range(ncores):
                pad = P - core_idx[k].size
                core_idx[k] = np.concatenate([core_idx[k], np.zeros(pad, np.int16)])
                core_row[k] = np.concatenate([core_row[k], np.full(pad, SENT, np.float32)])
                core_deg[k] = np.concatenate([core_deg[k], np.zeros(pad, np.float32)])
                core_til[k] = np.concatenate([core_til[k], np.full(pad, -1, np.int32)])
                idx_streams[k].append(np.ascontiguousarray(core_idx[k].reshape(-1, 16).T))
            g.calls.append(Call(pass_id=c, idx_ofs=idx_cursor, n_idx=P))
            idx_cursor += P // 16
            for j in range(nch):
                sl = slice(j * 128, (j + 1) * 128)
                tiles_here = set()
                for k in range(ncores):
                    th = core_til[k][sl]
                    tiles_here.update(np.unique(th[th >= 0]).tolist())
                for t in sorted(tiles_here):
                    rmin, rmax = 128, -1
                    for k in range(ncores):
                        mk = core_til[k][sl] == t
                        if mk.any():
                            rk = core_row[k][sl][mk]
                            rmin = min(rmin, int(rk.min()))
                            rmax = max(rmax, int(rk.max()))
                    r0 = rmin
                    while r0 <= rmax:
                        w = min(WIN, 128 - r0)
                        cols = []
                        any_core = False
                        for k in range(ncores):
                            rj = core_row[k][sl]
                            mw = ((core_til[k][sl] == t) & (rj >= r0)
                                  & (rj < r0 + w))
                            rc = np.full(128, SENT, np.float32)
                            dc = np.zeros(128, np.float32)
                            if mw.any():
                                any_core = True
                                rc[mw] = rj[mw] - r0
                                dc[mw] = core_deg[k][sl][mw]
                            cols.append((rc, dc))
                        if any_core:
                            tile_seglists[t].append(
                                (Seg(pass_id=c, chunk=j, r0=r0, w=w), cols))
                        r0 += w
        g.idx_w = idx_cursor - g.idx_ofs
        for t in g.tiles:
            segl = []
            for seg, cols in tile_seglists[t]:
                seg.scol = scol
                scol += 1
                for k in range(ncores):
                    row_cols[k].append(cols[k][0])
                    deg_cols[k].append(cols[k][1])
                segl.append(seg)
            g.segs[t] = segl
        groups.append(g)

    arrays = []
    for k in range(ncores):
        idx_dram = (np.concatenate(idx_streams[k], axis=1)
                    if idx_streams[k] else np.zeros((16, 1), np.int16))
        idx_dram = np.ascontiguousarray(np.tile(idx_dram, (8, 1)))
        rowloc = (np.stack(row_cols[k], axis=1).astype(np.float16)
                  if row_cols[k] else np.full((128, 1), SENT, np.float16))
        degseg = (np.stack(deg_cols[k], axis=1).astype(np.float16)
                  if deg_cols[k] else np.zeros((128, 1), np.float16))
        arrays.append(dict(idx=idx_dram, rowloc=rowloc, degseg=degseg))

    meta = dict(groups=groups, n_tiles=n_tiles, nsub=nsub, sub=sub,
                idx_w=max(idx_cursor, 1), n_segs=max(scol, 1),
                pass_chunks=max_pass_chunks,
                max_iw=max((g.idx_w for g in groups), default=1),
                max_tile_segs=max((len(s) for g in groups
                                   for s in g.segs.values()), default=1))
    return meta, arrays


# ---------------------------------------------------------------------------
# Device program
# ---------------------------------------------------------------------------


def build_gcn(meta, n_nodes, d=128, g_bufs=2, i_bufs=4, s_bufs=6, r_bufs=4, p_bufs=3, num_devices=NCORES, repeats=1):
    groups = meta["groups"]
    sub = meta["sub"]
    SMAXT = max(meta["max_tile_segs"], 1)

    nc = bacc.Bacc("TRN2", target_bir_lowering=False, debug=False,
                   num_devices=num_devices)

    x = nc.dram_tensor("x", [n_nodes, d], F16, kind="ExternalInput")
    w = nc.dram_tensor("w", [d, d], F16, kind="ExternalInput")
    idxd = nc.dram_tensor("idx", [128, meta["idx_w"]], I16, kind="ExternalInput")
    rowlocd = nc.dram_tensor("rowloc", [128, meta["n_segs"]], F16,
                             kind="ExternalInput")
    degsegd = nc.dram_tensor("degseg", [128, meta["n_segs"]], F16,
                             kind="ExternalInput")
    out = nc.dram_tensor("out", [meta["n_tiles"] * 128, d], F16,
                         kind="ExternalOutput")

    with tile.TileContext(nc) as tc:
        with (
            tc.tile_pool(name="static", bufs=1) as spool,
            tc.tile_pool(name="g", bufs=g_bufs) as gpool,
            tc.tile_pool(name="idxp", bufs=i_bufs) as ipool,
            tc.tile_pool(name="rhs", bufs=r_bufs) as rpool,
            tc.tile_pool(name="psum", bufs=p_bufs, space="PSUM") as ppool,
            tc.tile_pool(name="small", bufs=s_bufs) as smpool,
        ):
            rowloc_sb = spool.tile([128, meta["n_segs"]], F16)
            nc.sync.dma_start(rowloc_sb[:, :], rowlocd[:, :])
            degseg_sb = spool.tile([128, meta["n_segs"]], F16)
            nc.sync.dma_start(degseg_sb[:, :], degsegd[:, :])
            w_sb = spool.tile([128, d], F16)
            nc.sync.dma_start(w_sb[:, :], w[:, :])
            zero_sb = spool.tile([128, d], F16)
            nc.vector.memset(zero_sb[:, :], 0.0)
            iota_sb = spool.tile([128, SMAXT * WIN], F16)
            nc.gpsimd.iota(iota_sb[:, :], pattern=[[0, SMAXT], [1, WIN]],
                           base=0, channel_multiplier=0,
                           allow_small_or_imprecise_dtypes=True)

            for _rep in range(repeats):
                for g in groups:
                    slabs = {}
                    for c in g.calls:
                        slabs[c.pass_id] = gpool.tile(
                            [128, meta["pass_chunks"][c.pass_id] * 128], F16,
                            name=f"gslab{c.pass_id}", tag=f"g{c.pass_id}")
                    islab = ipool.tile([128, meta["max_iw"]], I16, tag="i")
                    if g.idx_w:
                        nc.sync.dma_start(islab[:, :g.idx_w],
                                          idxd[:, g.idx_ofs:g.idx_ofs + g.idx_w])
                    for c in g.calls:
                        nch = c.n_idx // 128
                        out_ap = slabs[c.pass_id][:, :nch * 128]
                        lofs = c.idx_ofs - g.idx_ofs
                        nc.gpsimd.dma_gather(
                            out_ap=out_ap.rearrange("p (c d) -> p c d", d=d),
                            in_ap=x[c.pass_id * sub:
                                    min((c.pass_id + 1) * sub, n_nodes), :],
                            idxs_ap=islab[:, lofs:lofs + c.n_idx // 16],
                            num_idxs=c.n_idx,
                            num_idxs_reg=c.n_idx,
                            elem_size=d,
                            single_packet=False,
                        )
                    for t in g.tiles:
                        segs = g.segs.get(t, [])
                        ns = len(segs)
                        psum_t = ppool.tile([128, 128], F32, space="PSUM",
                                            tag="pT")
                        nc.tensor.matmul(out=psum_t[:, :], lhsT=zero_sb[:, :],
                                         rhs=zero_sb[:, :], start=True,
                                         stop=(ns == 0), skip_group_check=True)
                        if ns:
                            s0 = segs[0].scol
                            rhs = rpool.tile([128, ns * WIN], F16, tag="rhs")
                            nc.vector.tensor_tensor(
                                out=rhs[:, :],
                                in0=rowloc_sb[:, s0:s0 + ns].to_broadcast(
                                    [128, ns, WIN]),
                                in1=iota_sb[:, :ns * WIN],
                                op=mybir.AluOpType.is_equal,
                            )
                            rhs2 = rpool.tile([128, ns * WIN], F16, tag="rhs2")
                            nc.vector.tensor_tensor(
                                out=rhs2[:, :],
                                in0=rhs[:, :],
                                in1=degseg_sb[:, s0:s0 + ns].to_broadcast(
                                    [128, ns, WIN]),
                                op=mybir.AluOpType.mult,
                            )
                            for i, s in enumerate(segs):
                                nc.tensor.matmul(
                                    out=psum_t[:, s.r0:s.r0 + s.w],
                                    lhsT=slabs[s.pass_id][:, s.chunk * 128:
                                                          (s.chunk + 1) * 128],
                                    rhs=rhs2[:, i * WIN:i * WIN + s.w],
                                    start=False, stop=(i == ns - 1),
                                    skip_group_check=True,
                                )
                        yt = smpool.tile([128, 128], F16, tag="yt")
                        nc.vector.tensor_copy(yt[:, :], psum_t[:, :])
                        psum_o = ppool.tile([128, 128], F32, space="PSUM",
                                            tag="pO")
                        nc.tensor.matmul(out=psum_o[:, :], lhsT=yt[:, :],
                                         rhs=w_sb[:, :], start=True, stop=True)
                        o = smpool.tile([128, 128], F16, tag="o")
                        nc.vector.tensor_copy(o[:, :], psum_o[:, :])
                        nc.sync.dma_start(out[t * 128:(t + 1) * 128, :], o[:, :])

    nc.compile()
    return nc


# ---------------------------------------------------------------------------
# Entry point
# ---------------------------------------------------------------------------

_CACHE = {}


def _get_program_and_arrays(X, weights, row_pointers, column_index, degrees,
                            repeats=1):
    n_nodes, d = X.shape
    cores, rows_per_core, n_total_rows = shard_edges(
        row_pointers, column_index, degrees, NCORES)
    meta, arrays = prep_all(cores, rows_per_core, n_nodes, gt=GT, nsub=NSUB)
    nc = build_gcn(meta, n_nodes, d, num_devices=NCORES, repeats=repeats)
    return nc, meta, arrays, rows_per_core, n_total_rows


def kernel(X, weights, row_pointers, column_index, degrees):
    X = np.asarray(X)
    weights = np.asarray(weights)
    row_pointers = np.asarray(row_pointers)
    column_index = np.asarray(column_index)
    degrees = np.asarray(degrees)
    n_nodes, d = X.shape
    assert d == 128 and weights.shape == (128, 128)

    nc, meta, arrays, rows_per_core, n_total_rows = _get_program_and_arrays(
        X, weights, row_pointers, column_index, degrees)

    Xf = np.ascontiguousarray(X.astype(np.float16))
    Wf = np.ascontiguousarray(weights.astype(np.float16))
    in_maps = [{"x": Xf, "w": Wf, **arrays[k]} for k in range(NCORES)]
    try:
        res = run_bass_kernel_spmd(nc, in_maps, list(range(NCORES)), trace=False)
    except Exception:
        import time as _time
        _time.sleep(30)
        res = run_bass_kernel_spmd(nc, in_maps, list(range(NCORES)), trace=False)

    pieces = []
    for k in range(NCORES):
        r_lo = min(k * rows_per_core, n_total_rows)
        r_hi = min(r_lo + rows_per_core, n_total_rows)
        pieces.append(res.results[k]["out"][:r_hi - r_lo])
    return np.concatenate(pieces).astype(np.float16)



# revision 19
# speedup vs baseline: 1.0061x; 1.0061x over previous
"""GCNConv Trainium2 kernel: out = A_norm @ (X @ W) == (A_norm @ X) @ W.

Hybrid dual-device gather design. The SpMM's per-edge source-row gather is the
bottleneck; it is split across two independent devices per core:

  - DMA side (cols >= C_CUT): per-edge dma_gather of 256B rows from DRAM X
    (2 int16 subtables), segment-sum on the PE via one-hot rhs matmuls into
    per-group [128, 512] fp32 PSUM quads, copied into an SBUF Y_dma^T slab.
  - Pool side (cols < C_CUT): X^T subtables are built in SBUF with
    dma_gather(transpose=True) at 4KB descriptors, then per-edge gathers run
    on the GPSIMD engine via ap_gather (uint32 cells = fp16 node pairs).
    Gathered G^T chunks are PE-transposed (parity-strided fp16 APs) into PSUM,
    batch-copied to SBUF by the Activation engine, segment-summed into PSUM
    quads, and accumulated into an SBUF Y_pool^T slab across passes.

Final per tile: yt = Y_dma^T + Y_pool^T (DVE), out_tile = yt.T @ W (PE),
Activation copies PSUM->SBUF, DMA writes out.

The program is rebuilt per input but identical across the 8 cores: all
data-dependent stream lengths are padded to the max over cores and segment
windows are unions over cores.
"""

import numpy as np
from dataclasses import dataclass, field

import concourse.bass as bass
import concourse.bacc as bacc
import concourse.tile as tile
from concourse import mybir
from concourse.bass_utils import run_bass_kernel_spmd
from concourse.vector_clock import ScopedClock

F16 = mybir.dt.float16
F32 = mybir.dt.float32
I16 = mybir.dt.int16
U32 = mybir.dt.uint32

NCORES = 8
WIN = 16          # psum column window per segment
SENT = 1024.0     # rowloc sentinel (never equals iota 0..WIN-1)
GT = 4            # tiles per group/quad
SMAXT = 32        # max segs per rhs-build batch
NSUB_P = 4        # pool-side X^T passes
CALLMAX = 6400    # pool slots per ap_gather call (multiple of 128)
STREAMW = 1536    # rowloc/degseg SBUF window (segs)

# ---------------------------------------------------------------------------
# Patch TileContext for walrus builds that reject >1 sync-wait/instruction.
# ---------------------------------------------------------------------------

_orig_commit = tile.TileContext._commit_instruction


def _commit_patched(self, inst, lazy_reg_writes: bool = True):
    si = getattr(inst, "sync_info", None)
    if (si is not None and si.on_wait and len(si.on_wait) > 1
            and inst.engine != mybir.EngineType.Unassigned):
        waits = list(si.on_wait)
        imm = [w for w in waits if w.wait_mode == "sem-ge-imm"]
        other = [w for w in waits if w.wait_mode != "sem-ge-imm"]
        assert len(other) <= 1, f"cannot split reg-waits: {waits}"
        keep = other if other else imm[:1]
        hoist = imm if other else imm[1:]
        inst.sync_info = mybir.SyncInfo(on_wait=list(keep),
                                        on_update=list(si.on_update or []))
        for w in hoist:
            nop = mybir.InstNoOp(name=self.nc.get_next_instruction_name(),
                                 ins=[], outs=[])
            nop.engine = inst.engine
            nop.bass_nofuse = True
            nop.sync_info = mybir.SyncInfo(on_wait=[w], on_update=[])
            _orig_commit(self, nop, lazy_reg_writes=False)
    return _orig_commit(self, inst, lazy_reg_writes)


def _drain_and_barrier_patched(self, tick_clock, wait_clock):
    nc = self.nc
    probe = nc.sync.nop(nofuse=True)
    wait_clock.add_sem_waits(probe.ins, ScopedClock({None: tick_clock.global_clock}))
    si = probe.ins.sync_info
    waits = list(si.on_wait) if si is not None and si.on_wait else []
    if waits:
        probe.ins.sync_info = mybir.SyncInfo(on_wait=waits[:1], on_update=[])
        for w in waits[1:]:
            n = nc.sync.nop(nofuse=True)
            n.ins.sync_info = mybir.SyncInfo(on_wait=[w], on_update=[])
    nc.sync.drain()
    nc.all_engine_barrier()
    assert self.sems is not None
    popped = nc._tile_sem_poison_stack.pop()
    assert popped is self._sem_poison
    nc.clear_and_free_semaphores(list(self.sems.allocated().values()))
    nc.all_engine_barrier()


tile.TileContext._commit_instruction = _commit_patched
tile.TileContext._drain_and_barrier = _drain_and_barrier_patched

# ---------------------------------------------------------------------------
# Host-side prep
# ---------------------------------------------------------------------------


@dataclass
class Seg:
    scol: int         # column in rowloc/degseg streams
    r0: int           # window start row within tile
    w: int            # window width
    tloc: int         # tile index within quad (0..GT-1)
    lhs_kind: str     # "dma" | "pool"
    lhs_ref: tuple    # dma: (pass_q, chunk) ; pool: (chunk, half)


@dataclass
class DmaCall:
    pass_q: int
    idx_ofs: int      # in dma idx stream (units of 16 idxs)
    n_idx: int


@dataclass
class DmaGroup:
    quad: int
    calls: list = field(default_factory=list)
    idx_ofs: int = 0
    idx_w: int = 0
    segs: list = field(default_factory=list)   # Segs, ordered by tile
    seg_cols: list = field(default_factory=list)


@dataclass
class PoolCall:
    s0: int           # slot range [s0, s1) within pass stream
    s1: int
    idx_ofs: int      # in pool idx stream (units of 16)
    chunks: list = field(default_factory=list)      # chunk ids in this call


@dataclass
class PoolPass:
    p: int
    base: int
    cnt: int
    gpad: int         # padded 16-node group count (transpose num_idxs)
    nslots: int
    calls: list = field(default_factory=list)
    chunk_halves: dict = field(default_factory=dict)  # chunk -> [halves]
    quad_segs: dict = field(default_factory=dict)     # quad -> [Seg]
    unit_of: dict = field(default_factory=dict)       # (chunk, half) -> idx
    quad_last_unit: dict = field(default_factory=dict)  # quad -> unit idx
    quad_seg_cols: dict = field(default_factory=dict)


def shard_edges(row_pointers, column_index, degrees, ncores=NCORES):
    rp = row_pointers.astype(np.int64)
    n_total_rows = rp.shape[0] - 1
    rows_per_core = (n_total_rows + ncores - 1) // ncores
    n_edges = column_index.shape[0]
    edge_row = np.minimum(
        np.searchsorted(rp[1:], np.arange(n_edges), side="right"),
        n_total_rows - 1)
    cores = []
    for r in range(ncores):
        r_lo = min(r * rows_per_core, n_total_rows)
        r_hi = min(r_lo + rows_per_core, n_total_rows)
        e_lo, e_hi = np.searchsorted(edge_row, [r_lo, r_hi])
        cores.append(((edge_row[e_lo:e_hi] - r_lo).astype(np.int64),
                      column_index[e_lo:e_hi].astype(np.int64),
                      degrees[e_lo:e_hi].astype(np.float32)))
    return cores, rows_per_core, n_total_rows


def _windows(rows):
    """Union window list [(r0, w)] of width<=WIN covering `rows` (sliding
    from rmin, skipping empty windows)."""
    rows = np.asarray(rows, dtype=np.int64)
    rmin, rmax = int(rows.min()), int(rows.max())
    out = []
    r0 = rmin
    while r0 <= rmax:
        w = min(WIN, 128 - r0)
        if ((rows >= r0) & (rows < r0 + w)).any():
            out.append((r0, w))
        r0 += w
    return out


def _wrap16(a, reps=8):
    w = np.ascontiguousarray(np.asarray(a, np.int16).reshape(-1, 16).T)
    return np.ascontiguousarray(np.tile(w, (reps, 1)))


class StreamAlloc:
    """Shared rowloc/degseg column allocator across both sides.

    Segs are created with scol=-1 plus their per-core columns; `finalize`
    assigns contiguous scols to a seg list (one rhs-batch-friendly run)."""

    def __init__(self, ncores):
        self.ncores = ncores
        self.row_cols = [[] for _ in range(ncores)]
        self.deg_cols = [[] for _ in range(ncores)]
        self.n = 0

    def finalize(self, seg_cols):
        """seg_cols: list of (Seg, cols); assigns sequential scols."""
        for seg, cols in seg_cols:
            for k in range(self.ncores):
                self.row_cols[k].append(cols[k][0])
                self.deg_cols[k].append(cols[k][1])
            seg.scol = self.n
            self.n += 1


def _batches_of(pp, call):
    work = [(j, h) for j in call.chunks for h in pp.chunk_halves[j]]
    return [work[i:i + 4] for i in range(0, len(work), 4)]


def prep_all(cores_edges, n_rows_core, n_nodes, c_cut):
    ncores = len(cores_edges)
    n_tiles = (n_rows_core + 127) // 128
    nq = (n_tiles + GT - 1) // GT

    # --- geometry ---
    pcnt = c_cut // NSUB_P
    assert pcnt % 16 == 0 and pcnt * NSUB_P == c_cut
    gpad = ((pcnt // 16 + 127) // 128) * 128
    ncells = 8 * gpad  # uint32 cells per pass slab
    assert ncells <= 32768
    dma_lo = c_cut
    dma_n = n_nodes - dma_lo
    sub_d = (dma_n + 1) // 2
    assert sub_d <= 32767, (c_cut, sub_d)
    dma_bases = [dma_lo, dma_lo + sub_d]
    dma_sizes = [sub_d, dma_n - sub_d]

    alloc = StreamAlloc(ncores)

    # --- per-core edge arrays split ---
    per_core = []
    for er, ec, ed in cores_edges:
        til = er // 128
        row = er % 128
        pool_m = ec < c_cut
        per_core.append(dict(til=til, row=row, col=ec, deg=ed, pool=pool_m))

    # =======================================================================
    # Pool side
    # =======================================================================
    pool_passes = []
    pidx_streams = [[] for _ in range(ncores)]
    pidx_cursor = 0

    for p in range(NSUB_P):
        base = p * pcnt
        pp = PoolPass(p=p, base=base, cnt=pcnt, gpad=gpad, nslots=0)

        # per (core, quad, half): edge lists ordered (tile, row)
        seglists = {}
        core_qh = []
        for k in range(ncores):
            d = per_core[k]
            m = d["pool"] & (d["col"] >= base) & (d["col"] < base + pcnt)
            ln = d["col"][m] - base
            a, g = ln % 16, ln // 16
            fpos = a * gpad + g
            cell = fpos // 2
            half = fpos % 2
            til, row, deg = d["til"][m], d["row"][m], d["deg"][m]
            quad = til // GT
            order = np.lexsort((row, til, half, quad))
            core_qh.append(dict(cell=cell[order], half=half[order],
                                til=til[order], row=row[order],
                                deg=deg[order], quad=quad[order]))

        # common run lengths per (quad, half)
        runs = []  # (quad, half, length)
        for q in range(nq):
            for h in (0, 1):
                L = 0
                for k in range(ncores):
                    cq = core_qh[k]
                    L = max(L, int(((cq["quad"] == q) & (cq["half"] == h)).sum()))
                if L:
                    runs.append((q, h, L))
        total = sum(L for _, _, L in runs)
        nslots = ((total + 127) // 128) * 128
        if nslots == 0:
            pool_passes.append(pp)
            continue
        pp.nslots = nslots

        # build padded common-layout slot arrays per core
        slot_cell = np.zeros((ncores, nslots), np.int16)
        slot_til = np.full((ncores, nslots), -1, np.int64)
        slot_row = np.zeros((ncores, nslots), np.int64)
        slot_deg = np.zeros((ncores, nslots), np.float32)
        slot_half = np.zeros(nslots, np.int64)   # common across cores
        slot_quad = np.full(nslots, -1, np.int64)
        ofs = 0
        for q, h, L in runs:
            slot_half[ofs:ofs + L] = h
            slot_quad[ofs:ofs + L] = q
            for k in range(ncores):
                cq = core_qh[k]
                mk = (cq["quad"] == q) & (cq["half"] == h)
                n_k = int(mk.sum())
                sl = slice(ofs, ofs + n_k)
                slot_cell[k, sl] = cq["cell"][mk]
                slot_til[k, sl] = cq["til"][mk]
                slot_row[k, sl] = cq["row"][mk]
                slot_deg[k, sl] = cq["deg"][mk]
            ofs += L

        # chunks & segs
        nch = nslots // 128
        quad_seg_cols = {}  # quad -> [(Seg, cols)]
        for j in range(nch):
            sl = slice(j * 128, (j + 1) * 128)
            halves = sorted(set(slot_half[sl].tolist()))
            pp.chunk_halves[j] = halves
            # segs per (half, tile) with union windows
            for h in halves:
                hm = slot_half[sl] == h
                tiles_here = set()
                for k in range(ncores):
                    tk = slot_til[k, sl]
                    tiles_here.update(np.unique(tk[hm & (tk >= 0)]).tolist())
                for t in sorted(tiles_here):
                    rows_u = []
                    for k in range(ncores):
                        mk = hm & (slot_til[k, sl] == t)
                        if mk.any():
                            rows_u.append(slot_row[k, sl][mk])
                    for r0, w in _windows(np.concatenate(rows_u)):
                        cols = []
                        for k in range(ncores):
                            rj = slot_row[k, sl]
                            mw = (hm & (slot_til[k, sl] == t)
                                  & (rj >= r0) & (rj < r0 + w))
                            rc = np.full(128, SENT, np.float32)
                            dc = np.zeros(128, np.float32)
                            rc[mw] = rj[mw] - r0
                            dc[mw] = slot_deg[k, sl][mw]
                            cols.append((rc, dc))
                        q = t // GT
                        seg = Seg(scol=-1, r0=r0, w=w, tloc=t % GT,
                                  lhs_kind="pool", lhs_ref=(j, h))
                        quad_seg_cols.setdefault(q, []).append((seg, cols))

        # defer scol assignment to schedule construction
        pp.quad_seg_cols = quad_seg_cols
        for q in sorted(quad_seg_cols):
            pp.quad_segs[q] = [s for s, _ in quad_seg_cols[q]]

        # work-unit order within pass: (chunk, half) ascending; quad's last
        unit_of = {}
        for j in range(nch):
            for h in pp.chunk_halves[j]:
                unit_of[(j, h)] = len(unit_of)
        pp.unit_of = unit_of
        pp.quad_last_unit = {
            q: max(unit_of[s.lhs_ref] for s in segs)
            for q, segs in pp.quad_segs.items()}

        # calls (128-aligned slot ranges)
        ncalls = max(1, -(-nslots // CALLMAX))
        npc = -(-nch // ncalls)
        c0 = 0
        for ci in range(ncalls):
            c1 = min(c0 + npc, nch)
            if c1 <= c0:
                break
            call = PoolCall(s0=c0 * 128, s1=c1 * 128, idx_ofs=pidx_cursor)
            call.chunks = list(range(c0, c1))
            pidx_cursor += (c1 - c0) * 128 // 16
            for k in range(ncores):
                pidx_streams[k].append(
                    np.ascontiguousarray(
                        slot_cell[k, c0 * 128:c1 * 128].reshape(-1, 16).T))
            pp.calls.append(call)
            c0 = c1
        pool_passes.append(pp)

    # X^T build idx streams (same for all cores)
    xt_idx = []
    for pp in pool_passes:
        gi = np.full(pp.gpad, pp.base // 16, np.int16)
        ngr = pp.cnt // 16
        gi[:ngr] = pp.base // 16 + np.arange(ngr, dtype=np.int16)
        xt_idx.append(_wrap16(gi))
    xt_idx_dram = (np.concatenate(xt_idx, axis=1) if xt_idx
                   else np.zeros((128, 16), np.int16))

    # =======================================================================
    # DMA side (baseline structure, 2 subtables, quad psums)
    # =======================================================================
    groups = []
    didx_streams = [[] for _ in range(ncores)]
    didx_cursor = 0
    max_pass_chunks = [0, 0]

    # per (core, tile): dma edges in CSR order
    tile_edges = []
    for k in range(ncores):
        d = per_core[k]
        m = ~d["pool"]
        til, row, col, deg = (d["til"][m], d["row"][m], d["col"][m],
                              d["deg"][m])
        te = []
        for t in range(n_tiles):
            mt = til == t
            te.append((row[mt], col[mt], deg[mt]))
        tile_edges.append(te)

    for q in range(nq):
        g = DmaGroup(quad=q)
        tiles = list(range(q * GT, min((q + 1) * GT, n_tiles)))
        g.idx_ofs = didx_cursor
        tile_seglists = {t: [] for t in tiles}
        for c in range(2):
            core_idx, core_row, core_deg, core_til = [], [], [], []
            for k in range(ncores):
                si, sr, sd, st = [], [], [], []
                for t in tiles:
                    rl, cl, dl = tile_edges[k][t]
                    m = (cl >= dma_bases[c]) & (cl < dma_bases[c] + dma_sizes[c])
                    si.append((cl[m] - dma_bases[c]).astype(np.int16))
                    sr.append(rl[m].astype(np.float32))
                    sd.append(dl[m].astype(np.float32))
                    st.append(np.full(int(m.sum()), t, np.int32))
                core_idx.append(np.concatenate(si))
                core_row.append(np.concatenate(sr))
                core_deg.append(np.concatenate(sd))
                core_til.append(np.concatenate(st))
            P = max(ci.size for ci in core_idx)
            P = ((P + 127) // 128) * 128
            if P == 0:
                continue
            nchq = P // 128
            max_pass_chunks[c] = max(max_pass_chunks[c], nchq)
            for k in range(ncores):
                pad = P - core_idx[k].size
                core_idx[k] = np.concatenate([core_idx[k], np.zeros(pad, np.int16)])
                core_row[k] = np.concatenate([core_row[k], np.full(pad, SENT, np.float32)])
                core_deg[k] = np.concatenate([core_deg[k], np.zeros(pad, np.float32)])
                core_til[k] = np.concatenate([core_til[k], np.full(pad, -1, np.int32)])
                didx_streams[k].append(
                    np.ascontiguousarray(core_idx[k].reshape(-1, 16).T))
            g.calls.append(DmaCall(pass_q=c, idx_ofs=didx_cursor, n_idx=P))
            didx_cursor += P // 16
            for j in range(nchq):
                sl = slice(j * 128, (j + 1) * 128)
                tiles_here = set()
                for k in range(ncores):
                    th = core_til[k][sl]
                    tiles_here.update(np.unique(th[th >= 0]).tolist())
                for t in sorted(tiles_here):
                    rows_u = []
                    for k in range(ncores):
                        mk = core_til[k][sl] == t
                        if mk.any():
                            rows_u.append(core_row[k][sl][mk].astype(np.int64))
                    for r0, w in _windows(np.concatenate(rows_u)):
                        cols = []
                        for k in range(ncores):
                            rj = core_row[k][sl]
                            mw = ((core_til[k][sl] == t) & (rj >= r0)
                                  & (rj < r0 + w))
                            rc = np.full(128, SENT, np.float32)
                            dc = np.zeros(128, np.float32)
                            rc[mw] = rj[mw] - r0
                            dc[mw] = core_deg[k][sl][mw]
                            cols.append((rc, dc))
                        seg = Seg(scol=-1, r0=r0, w=w, tloc=t % GT,
                                  lhs_kind="dma", lhs_ref=(c, j))
                        tile_seglists[t].append((seg, cols))
        g.idx_w = didx_cursor - g.idx_ofs
        for t in tiles:
            g.seg_cols.extend(tile_seglists[t])
        g.segs = [s for s, _ in g.seg_cols]
        groups.append(g)

    # first/last pool pass touching each quad
    first_pass = {}
    last_pass = {}
    for pp in pool_passes:
        for q in pp.quad_segs:
            first_pass.setdefault(q, pp.p)
            last_pass[q] = pp.p

    # ---- emission schedule (shared by scol finalize and device builder) ----
    pool_units = []
    for pp in pool_passes:
        if pp.nslots == 0:
            continue
        pool_units.append(("build", pp.p))
        for ci in range(len(pp.calls)):
            pool_units.append(("call", pp.p, ci))
    n_units = len(pool_units)
    dma_sched = {}
    if n_units > 1:
        lastu = max(1, int(n_units * (NSUB_P - 1) / NSUB_P))
        for g in range(nq):
            u = min(int(g * lastu / max(nq - 1, 1)), n_units - 1)
            dma_sched.setdefault(u, []).append(g)
    else:
        dma_sched[0] = list(range(nq))

    schedule = []
    dma_done = set()
    fins_done = set()

    def sched_dma(g):
        if g not in dma_done:
            dma_done.add(g)
            schedule.append(("dma", g))
            alloc.finalize(groups[g].seg_cols)

    def sched_fin(q):
        if q not in fins_done:
            sched_dma(q)
            fins_done.add(q)
            schedule.append(("fin", q))

    by_pass = {pp.p: pp for pp in pool_passes}
    pending = []
    for ui, unit in enumerate(pool_units):
        if unit[0] == "build":
            pp = by_pass[unit[1]]
            schedule.append(("build", pp.p))
            pending = sorted(pp.quad_last_unit,
                             key=lambda q: pp.quad_last_unit[q])
        else:
            _, p, ci = unit
            pp = by_pass[p]
            call = pp.calls[ci]
            schedule.append(("gather", p, ci))
            for bi, batch in enumerate(_batches_of(pp, call)):
                schedule.append(("tbatch", p, ci, bi))
                u = pp.unit_of[batch[-1]]
                while pending and pp.quad_last_unit[pending[0]] <= u:
                    q = pending.pop(0)
                    schedule.append(("quad", p, q))
                    alloc.finalize(pp.quad_seg_cols[q])
                    if last_pass.get(q) == p:
                        sched_fin(q)
        for g in dma_sched.get(ui, []):
            sched_dma(g)
    for q in range(nq):
        sched_fin(q)

    # =======================================================================
    # Pack arrays
    # =======================================================================
    arrays = []
    for k in range(ncores):
        didx = (np.concatenate(didx_streams[k], axis=1)
                if didx_streams[k] else np.zeros((16, 1), np.int16))
        didx = np.ascontiguousarray(np.tile(didx, (8, 1)))
        pidx = (np.concatenate(pidx_streams[k], axis=1)
                if pidx_streams[k] else np.zeros((16, 1), np.int16))
        pidx = np.ascontiguousarray(np.tile(pidx, (8, 1)))
        rowloc = (np.stack(alloc.row_cols[k], axis=1).astype(np.float16)
                  if alloc.row_cols[k] else np.full((128, 1), SENT, np.float16))
        degseg = (np.stack(alloc.deg_cols[k], axis=1).astype(np.float16)
                  if alloc.deg_cols[k] else np.zeros((128, 1), np.float16))
        arrays.append(dict(didx=didx, pidx=pidx, xtidx=xt_idx_dram,
                           rowloc=rowloc, degseg=degseg))

    meta = dict(groups=groups, pool_passes=pool_passes, n_tiles=n_tiles,
                nq=nq, gpad=gpad, ncells=ncells,
                dma_bases=dma_bases, dma_sizes=dma_sizes,
                didx_w=max(didx_cursor, 1), pidx_w=max(pidx_cursor, 1),
                n_segs=max(alloc.n, 1), pass_chunks=max_pass_chunks,
                max_diw=max((g.idx_w for g in groups), default=1),
                first_pass=first_pass, schedule=schedule)
    return meta, arrays


# ---------------------------------------------------------------------------
# Device program
# ---------------------------------------------------------------------------


def build_gcn(meta, n_nodes, d=128, num_devices=NCORES):
    groups = meta["groups"]
    pool_passes = meta["pool_passes"]
    nq = meta["nq"]
    n_tiles = meta["n_tiles"]
    gpad = meta["gpad"]
    ncells = meta["ncells"]
    first_pass = meta["first_pass"]
    NT128 = n_tiles * 128

    nc = bacc.Bacc("TRN2", target_bir_lowering=False, debug=False,
                   num_devices=num_devices)

    x = nc.dram_tensor("x", [n_nodes, d], F16, kind="ExternalInput")
    w = nc.dram_tensor("w", [d, d], F16, kind="ExternalInput")
    didxd = nc.dram_tensor("didx", [128, meta["didx_w"]], I16, kind="ExternalInput")
    pidxd = nc.dram_tensor("pidx", [128, meta["pidx_w"]], I16, kind="ExternalInput")
    xtidxd = nc.dram_tensor("xtidx", [128, max(gpad // 16 * len(pool_passes), 1)],
                            I16, kind="ExternalInput")
    rowlocd = nc.dram_tensor("rowloc", [128, meta["n_segs"]], F16,
                             kind="ExternalInput")
    degsegd = nc.dram_tensor("degseg", [128, meta["n_segs"]], F16,
                             kind="ExternalInput")
    out = nc.dram_tensor("out", [NT128, d], F16, kind="ExternalOutput")

    x16 = x[:, :].rearrange("(a b) e -> a (b e)", b=16)  # 16-node groups

    with tile.TileContext(nc) as tc:
        with (
            tc.tile_pool(name="static", bufs=1) as spool,
            tc.tile_pool(name="xt", bufs=1) as xtpool,
            tc.tile_pool(name="gslab", bufs=2) as gpool,
            tc.tile_pool(name="dslab", bufs=2) as dpool,
            tc.tile_pool(name="idxp", bufs=2) as ipool,
            tc.tile_pool(name="pidxp", bufs=2) as pipool,
            tc.tile_pool(name="stage", bufs=4) as stpool,
            tc.tile_pool(name="rhs", bufs=2) as rpool,
            tc.tile_pool(name="small", bufs=2) as smpool,
            tc.tile_pool(name="stream", bufs=2) as strpool,
            tc.tile_pool(name="tp", bufs=2, space="PSUM") as tppool,
            tc.tile_pool(name="pq", bufs=2, space="PSUM") as pqpool,
            tc.tile_pool(name="dq", bufs=2, space="PSUM") as dqpool,
            tc.tile_pool(name="po", bufs=2, space="PSUM") as popool,
        ):
            # ---- statics ----
            w_sb = spool.tile([128, d], F16)
            nc.sync.dma_start(w_sb[:, :], w[:, :])
            xtidx_sb = spool.tile([128, max(gpad // 16 * len(pool_passes), 1)], I16)
            nc.sync.dma_start(xtidx_sb[:, :], xtidxd[:, :])
            zero_sb = spool.tile([128, GT * 128], F16)
            nc.vector.memset(zero_sb[:, :], 0.0)
            iota_sb = spool.tile([128, SMAXT * WIN], F16)
            nc.gpsimd.iota(iota_sb[:, :], pattern=[[0, SMAXT], [1, WIN]],
                           base=0, channel_multiplier=0,
                           allow_small_or_imprecise_dtypes=True)
            io_f = spool.tile([128, 128], F16)
            nc.gpsimd.iota(io_f[:, :], pattern=[[1, 128]], base=0,
                           channel_multiplier=0,
                           allow_small_or_imprecise_dtypes=True)
            io_p = spool.tile([128, 128], F16)
            nc.gpsimd.iota(io_p[:, :], pattern=[[0, 128]], base=0,
                           channel_multiplier=1,
                           allow_small_or_imprecise_dtypes=True)
            ident = spool.tile([128, 128], F16)
            nc.vector.tensor_tensor(out=ident[:, :], in0=io_f[:, :],
                                    in1=io_p[:, :],
                                    op=mybir.AluOpType.is_equal)
            y_dma = spool.tile([128, NT128], F16)
            y_pool = spool.tile([128, NT128], F16)

            # quads never touched by the pool side: zero them once
            for q in range(nq):
                if q not in first_pass:
                    qw = min(GT * 128, NT128 - q * GT * 128)
                    nc.vector.memset(y_pool[:, q * GT * 128:q * GT * 128 + qw],
                                     0.0)

            # ---------------------------------------------------------------
            # emission helpers
            # ---------------------------------------------------------------
            def rhs_batches(segs):
                """Split seg list into contiguous-scol batches of <= SMAXT."""
                batches = []
                cur = []
                for s in segs:
                    if cur and (len(cur) >= SMAXT
                                or s.scol != cur[-1].scol + 1):
                        batches.append(cur)
                        cur = []
                    cur.append(s)
                if cur:
                    batches.append(cur)
                return batches

            stream_st = dict(tr=None, td=None, w0=0, end=0)

            def stream_tiles(s0, ns):
                if stream_st["tr"] is None or s0 + ns > stream_st["end"]:
                    cap = min(STREAMW, meta["n_segs"] - s0)
                    tr = strpool.tile([128, STREAMW], F16, name="strr",
                                      tag="sr")
                    td = strpool.tile([128, STREAMW], F16, name="strd",
                                      tag="sd")
                    nc.sync.dma_start(tr[:, :cap], rowlocd[:, s0:s0 + cap])
                    nc.sync.dma_start(td[:, :cap], degsegd[:, s0:s0 + cap])
                    stream_st.update(tr=tr, td=td, w0=s0, end=s0 + cap)
                o = s0 - stream_st["w0"]
                return stream_st["tr"], stream_st["td"], o

            def emit_segsum(qpsum, segs, lhs_of, qbase_tiles):
                """Zero quad psum, run seg matmuls, close accumulation."""
                qw = qbase_tiles * 128
                nc.tensor.matmul(out=qpsum[:, :qw], lhsT=zero_sb[:, 0:128],
                                 rhs=zero_sb[:, :qw], start=True,
                                 stop=(len(segs) == 0), skip_group_check=True)
                if not segs:
                    return
                batches = rhs_batches(segs)
                nseg_done = 0
                for b in batches:
                    ns = len(b)
                    s0 = b[0].scol
                    tr, td, o = stream_tiles(s0, ns)
                    rhs1 = rpool.tile([128, ns * WIN], F16, name="rhs1",
                                      tag="rhs1")
                    nc.vector.tensor_tensor(
                        out=rhs1[:, :],
                        in0=tr[:, o:o + ns].to_broadcast([128, ns, WIN]),
                        in1=iota_sb[:, :ns * WIN],
                        op=mybir.AluOpType.is_equal)
                    rhs2 = rpool.tile([128, ns * WIN], F16, name="rhs2",
                                      tag="rhs2")
                    nc.vector.tensor_tensor(
                        out=rhs2[:, :],
                        in0=rhs1[:, :],
                        in1=td[:, o:o + ns].to_broadcast([128, ns, WIN]),
                        op=mybir.AluOpType.mult)
                    for i, s in enumerate(b):
                        nseg_done += 1
                        nc.tensor.matmul(
                            out=qpsum[:, s.tloc * 128 + s.r0:
                                      s.tloc * 128 + s.r0 + s.w],
                            lhsT=lhs_of(s),
                            rhs=rhs2[:, i * WIN:i * WIN + s.w],
                            start=False, stop=(nseg_done == len(segs)),
                            skip_group_check=True)

            # ---- dma group emission ----
            def emit_dma_group(g):
                slabs = {}
                for c in g.calls:
                    slabs[c.pass_q] = dpool.tile(
                        [128, meta["pass_chunks"][c.pass_q] * 128], F16,
                        name=f"dslab{c.pass_q}", tag=f"d{c.pass_q}")
                islab = ipool.tile([128, meta["max_diw"]], I16, name="islab",
                                   tag="di")
                if g.idx_w:
                    nc.sync.dma_start(islab[:, :g.idx_w],
                                      didxd[:, g.idx_ofs:g.idx_ofs + g.idx_w])
                for c in g.calls:
                    nchq = c.n_idx // 128
                    out_ap = slabs[c.pass_q][:, :nchq * 128]
                    lofs = c.idx_ofs - g.idx_ofs
                    b = meta["dma_bases"][c.pass_q]
                    sz = meta["dma_sizes"][c.pass_q]
                    nc.gpsimd.dma_gather(
                        out_ap=out_ap.rearrange("p (c e) -> p c e", e=d),
                        in_ap=x[b:b + sz, :],
                        idxs_ap=islab[:, lofs:lofs + c.n_idx // 16],
                        num_idxs=c.n_idx,
                        num_idxs_reg=c.n_idx,
                        elem_size=d,
                        single_packet=False)
                qtiles = min(GT, n_tiles - g.quad * GT)
                qpsum = dqpool.tile([128, GT * 128], F32, space="PSUM",
                                    name="dq", tag="dq")

                def lhs_of(s):
                    cpass, j = s.lhs_ref
                    return slabs[cpass][:, j * 128:(j + 1) * 128]

                emit_segsum(qpsum, g.segs, lhs_of, qtiles)
                nc.scalar.copy(
                    y_dma[:, g.quad * GT * 128:g.quad * GT * 128 + qtiles * 128],
                    qpsum[:, :qtiles * 128])

            # ---- pool pass emission (generator of units) ----
            def emit_xt_build(pp):
                xt = xtpool.tile([128, 16 * gpad], F16, name="xt", tag="xt")
                nc.gpsimd.dma_gather(
                    out_ap=xt[:, :].rearrange("p (c e) -> p c e", e=gpad),
                    in_ap=x16,
                    idxs_ap=xtidx_sb[:, pp.p * (gpad // 16):
                                     (pp.p + 1) * (gpad // 16)],
                    num_idxs=gpad,
                    num_idxs_reg=gpad,
                    elem_size=16 * d,
                    transpose=True,
                    single_packet=False)
                return xt

            def emit_pool_gather(pp, call, xt):
                nsl = call.s1 - call.s0
                pidx = pipool.tile([128, CALLMAX // 16], I16, name="pidx",
                                   tag="pi")
                nc.sync.dma_start(pidx[:, :nsl // 16],
                                  pidxd[:, call.idx_ofs:call.idx_ofs + nsl // 16])
                gslab = gpool.tile([128, CALLMAX], U32, name="gslab", tag="g")
                nc.gpsimd.ap_gather(
                    out_ap=gslab[:, :nsl].rearrange("p (n e) -> p n e", e=1),
                    in_ap=xt[:, :].bitcast(U32).rearrange(
                        "p (n e) -> p n e", e=1),
                    idxs_ap=pidx[:, :nsl // 16],
                    channels=128,
                    num_elems=ncells,
                    d=1,
                    num_idxs=nsl)
                return gslab

            def emit_tbatch(pp, call, batch, gslab, stage_of):
                gf16 = gslab[:, :].bitcast(F16)  # [128, 2*CALLMAX]
                tp = tppool.tile([128, 4 * 128], F16, space="PSUM",
                                 name="tp", tag="tp")
                st = stpool.tile([128, 4 * 128], F16, name="st", tag="st")
                for i, (j, h) in enumerate(batch):
                    s0 = (j * 128 - call.s0)
                    src = gf16[:, 2 * s0 + h: 2 * s0 + h + 256].rearrange(
                        "p (n two) -> p n two", two=2)[:, :, 0]
                    nc.tensor.matmul(out=tp[:, i * 128:(i + 1) * 128],
                                     lhsT=src, rhs=ident[:, :],
                                     is_transpose=True,
                                     skip_group_check=True)
                nc.scalar.copy(st[:, :len(batch) * 128],
                               tp[:, :len(batch) * 128])
                for i, (j, h) in enumerate(batch):
                    stage_of[(j, h)] = (st, i)

            def emit_pool_quad(pp, q, stage_of):
                segs = pp.quad_segs.get(q, [])
                qtiles = min(GT, n_tiles - q * GT)
                qpsum = pqpool.tile([128, GT * 128], F32, space="PSUM",
                                    name="pq", tag="pq")

                def lhs_of(s):
                    st, i = stage_of[s.lhs_ref]
                    return st[:, i * 128:(i + 1) * 128]

                emit_segsum(qpsum, segs, lhs_of, qtiles)
                ysl = y_pool[:, q * GT * 128:q * GT * 128 + qtiles * 128]
                if first_pass.get(q) == pp.p:
                    nc.scalar.copy(ysl, qpsum[:, :qtiles * 128])
                else:
                    nc.vector.tensor_tensor(out=ysl, in0=qpsum[:, :qtiles * 128],
                                            in1=ysl, op=mybir.AluOpType.add)

            def emit_final(q):
                qtiles = min(GT, n_tiles - q * GT)
                po = popool.tile([128, GT * 128], F32, space="PSUM",
                                 name="po", tag="po")
                yt = smpool.tile([128, GT * 128], F16, name="yt", tag="yt")
                b0 = q * GT * 128
                nc.vector.tensor_tensor(out=yt[:, :qtiles * 128],
                                        in0=y_dma[:, b0:b0 + qtiles * 128],
                                        in1=y_pool[:, b0:b0 + qtiles * 128],
                                        op=mybir.AluOpType.add)
                for i in range(qtiles):
                    nc.tensor.matmul(out=po[:, i * 128:(i + 1) * 128],
                                     lhsT=yt[:, i * 128:(i + 1) * 128],
                                     rhs=w_sb[:, :], start=True, stop=True,
                                     skip_group_check=True)
                o = smpool.tile([128, GT * 128], F16, name="o", tag="o")
                nc.scalar.copy(o[:, :qtiles * 128], po[:, :qtiles * 128])
                for i in range(qtiles):
                    t = q * GT + i
                    nc.sync.dma_start(out[t * 128:(t + 1) * 128, :],
                                      o[:, i * 128:(i + 1) * 128])

            # ---------------------------------------------------------------
            # consume the prep-computed emission schedule
            # ---------------------------------------------------------------
            by_pass = {pp.p: pp for pp in pool_passes}
            stage_of = {}
            xt = None
            gslab = None
            for ent in meta["schedule"]:
                kind = ent[0]
                if kind == "build":
                    xt = emit_xt_build(by_pass[ent[1]])
                    stage_of = {}
                elif kind == "gather":
                    pp = by_pass[ent[1]]
                    gslab = emit_pool_gather(pp, pp.calls[ent[2]], xt)
                elif kind == "tbatch":
                    pp = by_pass[ent[1]]
                    call = pp.calls[ent[2]]
                    batch = _batches_of(pp, call)[ent[3]]
                    emit_tbatch(pp, call, batch, gslab, stage_of)
                elif kind == "quad":
                    emit_pool_quad(by_pass[ent[1]], ent[2], stage_of)
                elif kind == "dma":
                    emit_dma_group(groups[ent[1]])
                elif kind == "fin":
                    emit_final(ent[1])

    nc.compile()
    return nc


# ---------------------------------------------------------------------------
# Entry point
# ---------------------------------------------------------------------------


def _pick_c_cut(n_nodes):
    # pool side covers cols < C; dma side needs (n_nodes - C)/2 <= 32767
    c = 44544
    c = max(c, n_nodes - 2 * 32767)
    c = min(c, n_nodes)
    c = ((c + NSUB_P * 16 - 1) // (NSUB_P * 16)) * (NSUB_P * 16)
    return c


def _get_program_and_arrays(X, weights, row_pointers, column_index, degrees):
    n_nodes, d = X.shape
    cores, rows_per_core, n_total_rows = shard_edges(
        row_pointers, column_index, degrees, NCORES)
    c_cut = _pick_c_cut(n_nodes)
    meta, arrays = prep_all(cores, rows_per_core, n_nodes, c_cut)
    nc = build_gcn(meta, n_nodes, d, num_devices=NCORES)
    return nc, meta, arrays, rows_per_core, n_total_rows


def kernel(X, weights, row_pointers, column_index, degrees):
    X = np.asarray(X)
    weights = np.asarray(weights)
    row_pointers = np.asarray(row_pointers)
    column_index = np.asarray(column_index)
    degrees = np.asarray(degrees)
    n_nodes, d = X.shape
    assert d == 128 and weights.shape == (128, 128)

    nc, meta, arrays, rows_per_core, n_total_rows = _get_program_and_arrays(
        X, weights, row_pointers, column_index, degrees)

    Xf = np.ascontiguousarray(X.astype(np.float16))
    Wf = np.ascontiguousarray(weights.astype(np.float16))
    in_maps = [{"x": Xf, "w": Wf, **arrays[k]} for k in range(NCORES)]
    try:
        res = run_bass_kernel_spmd(nc, in_maps, list(range(NCORES)), trace=False)
    except Exception:
        import time as _time
        _time.sleep(30)
        res = run_bass_kernel_spmd(nc, in_maps, list(range(NCORES)), trace=False)

    pieces = []
    for k in range(NCORES):
        r_lo = min(k * rows_per_core, n_total_rows)
        r_hi = min(r_lo + rows_per_core, n_total_rows)
        pieces.append(res.results[k]["out"][:r_hi - r_lo])
    return np.concatenate(pieces).astype(np.float16)
